# revision 35
# baseline (speedup 1.0000x reference)
"""Trainium2 Bass kernel for nn_Block_71665824301263 (GNN message passing block).

Computation (see reference): BatchNorm -> TransformerConv-style edge attention
(6 heads, edge features added to K and V, segment softmax over incoming edges)
-> skip + residual -> BatchNorm -> MLP (gelu) -> residual.

Distribution over 8 NeuronCores:
- nodes sharded 2500/core; incoming edges partitioned by dst and sorted by dst
- weights replicated; BN1 is folded into the q/k/v/skip projection weights on
  the host (mean/var of the input x is a pure function of the input), so no
  BN1 stats pass or AllReduce is needed on device
- k/v projections AllGather'ed (fp8, in two node-chunks so the collective
  overlaps the q/skip projections); BN2 stats AllReduce'd
- q projection (pre-scaled by 1/sqrt(dhead)) written to DRAM and gathered
  per-edge by dst via batched SWDGE dma_gather (12 tiles = 1536 rows/instr)
- k/v rows gathered per-edge by src via batched dma_gather

Per-super (TF=3 tiles) attention dataflow:
  e = edge_attr @ We (PE, PSUM) -> e_sb bf16 (ACT copy)
  ve = v[src]+e accumulated in PSUM (PE idn-matmul on fp8 rows)
  ke = k[src]+e, prod = q[dst]*ke  (DVE bf16 2x mode)
  logits = reduce(prod by head) (DVE), w = exp (ACT, bf16)
  wve = ve*w (DVE, d-major/h-minor layout), agg += S^T @ [wve|w] (PE)
On block finalize the x+skip+attn rows are transposed (PE) into a
feature-major x2T with BN2 sums accumulated for free (ACT accum_out).
"""

import os
import numpy as np
import ml_dtypes

import concourse.bass as bass
import concourse.bacc as bacc
import concourse.tile as tile
import concourse.mybir as mybir
from concourse.bass_utils import run_bass_kernel_spmd
from concourse.masks import make_identity

C = 8            # cores
N = 20000        # nodes
NL = N // C      # nodes per core
D = 384
H = 6
DHEAD = 64
DH = 2 * D       # mlp hidden
P = 128
NBLK = (NL + P - 1) // P      # 20 node blocks per core (last has 68)
TF = 3                        # tiles per super (DVE fuse factor)
G = int(os.environ.get("KG", "12"))  # tiles per dma_gather chunk (multiple of TF)
SCALE = 1.0 / np.sqrt(np.float32(DHEAD))
EPS = 1e-5
KAG2 = int(os.environ.get("KAG2", "1"))  # chunked (2-piece) kv AllGather
SPLIT = 10 * P                # rows per core in AllGather chunk 1 (blocks 0-9)
REST = NL - SPLIT

f32 = mybir.dt.float32
bf16 = mybir.dt.bfloat16
f8 = mybir.dt.float8e4
i32 = mybir.dt.int32
i16 = mybir.dt.int16
BF = ml_dtypes.bfloat16
AF = mybir.ActivationFunctionType
ALU = mybir.AluOpType


def _block_pb(b):
    return min(P, NL - b * P)


def _remap_kv(g):
    """Global node id -> row in the chunk-AllGathered kv_full layout."""
    if not KAG2:
        return g
    c, r = g // NL, g % NL
    return np.where(r < SPLIT, c * SPLIT + r,
                    C * SPLIT + c * REST + (r - SPLIT))


def _wrap_idx(flat):
    """Flat index list -> dma_gather idxs layout [128, n/16] int16."""
    a = flat.astype(np.int16).reshape(-1, 16).T          # (i%16, i//16)
    return np.ascontiguousarray(np.tile(a, (8, 1)))


def _balance_nodes(dst):
    """Assign nodes to (core, block) slots balancing per-slot in-degree sums
    so the per-block tile counts K[b] (max over cores) are minimal.
    Returns newid[g] (permuted node id) and perm (orig id per new row)."""
    deg = np.bincount(dst, minlength=N)
    nslots = C * NBLK
    cap = np.full(nslots, P, np.int64)
    cap[NBLK - 1::NBLK] = NL - (NBLK - 1) * P          # last block: 68
    load = np.zeros(nslots, np.float64)
    fill = np.zeros(nslots, np.int64)
    order = np.argsort(-deg, kind="stable")
    slot_of = np.zeros(N, np.int64)
    import heapq
    heap = [(0.0, int(s)) for s in range(nslots)]
    heapq.heapify(heap)
    for g in order:
        while True:
            l, s = heapq.heappop(heap)
            if fill[s] < cap[s] and l == load[s]:
                break
        slot_of[g] = s
        fill[s] += 1
        load[s] += deg[g]
        if fill[s] < cap[s]:
            heapq.heappush(heap, (load[s], s))
    # new row within slot: order of assignment
    newid = np.zeros(N, np.int64)
    slot_base = np.zeros(nslots, np.int64)
    for s in range(nslots):
        c, b = divmod(s, NBLK)
        slot_base[s] = c * NL + b * P
    counter = np.zeros(nslots, np.int64)
    for g in order:
        s = slot_of[g]
        newid[g] = slot_base[s] + counter[s]
        counter[s] += 1
    perm = np.zeros(N, np.int64)
    perm[newid] = np.arange(N)
    return newid, perm


def _prep_host(x, edge_index, edge_attr, weights):
    """Shard + pad edges, build per-core input maps. Returns (in_maps, K, T)."""
    src = np.asarray(edge_index[0]).astype(np.int64)
    dst = np.asarray(edge_index[1]).astype(np.int64)
    x = np.asarray(x, dtype=np.float32)
    edge_attr = np.asarray(edge_attr, dtype=np.float32)

    newid, perm = _balance_nodes(dst)
    src = newid[src]
    dst = newid[dst]
    x = np.ascontiguousarray(x[perm])

    cores = []
    cnt = np.zeros((C, NBLK), np.int64)
    for c in range(C):
        sel = (dst >= c * NL) & (dst < (c + 1) * NL)
        eids = np.nonzero(sel)[0]
        d_loc = (dst[eids] - c * NL).astype(np.int64)
        order = np.argsort(d_loc, kind="stable")
        eids = eids[order]
        d_loc = d_loc[order]
        s_glob = src[eids]
        blk = d_loc // P
        cnt[c] = np.bincount(blk, minlength=NBLK)
        cores.append((eids, d_loc, s_glob, blk))

    K = [max(1, int(-(-cnt[:, b].max() // P))) for b in range(NBLK)]
    T = sum(K)
    pad = (-T) % G
    K[NBLK - 1] += pad
    T += pad
    tile_block = np.repeat(np.arange(NBLK), K)          # block id per tile
    blk_tile_start = np.concatenate([[0], np.cumsum(K)])[:NBLK]
    blk_edge_start = blk_tile_start * P

    # ---- fold BN1 (input-only statistics) into the projection weights:
    # h = (x - mu) * rsqrt(var + eps) * g1 + b1 = x * s1 + t1, so
    # h @ W + b = x @ (diag(s1) W) + (t1 @ W + b).
    (Wq, bq, Wk, bk, Wv, bv, We, Wskip, bskip,
     g1, b1, g2, b2, W1, bm1, W2, bm2) = weights
    x64 = x.astype(np.float64)
    mu1 = x64.mean(0)
    var1 = ((x64 - mu1) ** 2).mean(0)
    s1 = g1.astype(np.float64) / np.sqrt(var1 + EPS)
    t1 = b1.astype(np.float64) - mu1 * s1

    def fold(W, b, scale=1.0):
        Wf = (s1[:, None] * W.astype(np.float64)) * scale
        bf_ = (t1 @ W.astype(np.float64) + b.astype(np.float64)) * scale
        return Wf.astype(np.float32), bf_.astype(np.float32)

    Wq, bq = fold(Wq, bq, SCALE)     # fold 1/sqrt(dhead) into q projection
    Wk, bk = fold(Wk, bk)
    Wv, bv = fold(Wv, bv)
    Wskip, bskip = fold(Wskip, bskip)

    # permute head-dim columns to d-major/h-minor so the per-head broadcast
    # multiplies (wve, finalize) have unit inner stride on the DVE
    pdh = (np.arange(H)[None, :] * DHEAD
           + np.arange(DHEAD)[:, None]).reshape(-1)
    Wq, bq = Wq[:, pdh], bq[pdh]
    Wk, bk = Wk[:, pdh], bk[pdh]
    Wv, bv = Wv[:, pdh], bv[pdh]
    We = We[:, pdh]

    def chunks(w, nk):
        return np.stack([w[i * P:(i + 1) * P] for i in range(nk)]).astype(BF)

    def aug(w, b):
        a = np.zeros((4, P, w.shape[1]), np.float32)
        a[:3] = np.stack([w[i * P:(i + 1) * P] for i in range(3)])
        a[3, 0] = b
        return a.astype(BF)

    shared = {
        "Wq": aug(Wq, bq), "Wk": aug(Wk, bk), "Wv": aug(Wv, bv),
        "Wsk": aug(Wskip, bskip),
        "We": chunks(We, 3),
        "W1": aug(W1, bm1)[:3],                 # bias separately (bm1T)
        "bm1T": np.asarray(bm1, np.float32).reshape(H, P).T.copy(),
        "W2": chunks(W2, 6),
        "bm2": np.asarray(bm2, np.float32).reshape(1, D).astype(BF),
        "gb": np.stack([np.asarray(v, np.float32).reshape(3, P)
                        for v in (g1, b1, g2, b2)], axis=-1),  # [3, P, 4]
    }

    in_maps = []
    needs_b_all = []
    for c in range(C):
        eids, d_loc, s_glob, blk = cores[c]
        starts = np.searchsorted(blk, np.arange(NBLK))
        rank = np.arange(len(blk)) - starts[blk]
        pos = blk_edge_start[blk] + rank

        src_pad = np.zeros(T * P, np.int64)
        dst_pad = np.zeros(T * P, np.int64)
        valid = np.zeros(T * P, bool)
        src_pad[pos] = s_glob
        dst_pad[pos] = d_loc
        valid[pos] = True

        # Within each block, order edges by src AllGather band (band A =
        # local src row < SPLIT) so leading gather chunks only need the
        # first AG piece. Stable per-block counting sort by band.
        band = ((src_pad % NL) >= SPLIT).astype(np.int64)
        band[~valid] = 0
        tb = np.repeat(tile_block, P)
        order2 = np.lexsort((np.arange(T * P), band, tb))
        src_pad = src_pad[order2]
        dst_pad = dst_pad[order2]
        valid = valid[order2]
        band = band[order2]
        reorder = order2  # new position i holds old row order2[i]

        ea_pad = np.zeros((T * P, D), np.float32)
        ea_pad[pos] = edge_attr[eids]
        ea_pad = ea_pad[reorder]
        eaT = ea_pad.astype(BF).reshape(T, P, 3, P).transpose(0, 3, 2, 1)

        S = np.zeros((T * P, P), np.float32)
        S[np.nonzero(valid)[0], (dst_pad - tb * P)[valid]] = 1.0
        S = S.astype(BF).reshape(T, P, P)

        # per gather-chunk: does it touch band B (needs full AllGather)?
        needs_b = np.logical_or.reduce(
            (band * valid.astype(np.int64)).reshape(-1, G * P) > 0, axis=1)

        # combined [T, P, 4, P]: chunks 0-2 = eaT (partition=feature),
        # chunk 3 = S (partition=edge)
        eaS = np.concatenate([eaT, S[:, :, None, :]], axis=2)
        eaS = np.ascontiguousarray(eaS)

        kvidx = _wrap_idx(_remap_kv(np.where(valid, src_pad, 0)))
        qdidx = _wrap_idx(np.where(valid, dst_pad, 0))

        x_loc = np.ascontiguousarray(x[c * NL:(c + 1) * NL])
        xT_loc = np.ascontiguousarray(x_loc.T).astype(BF)

        m = {"x_loc": x_loc, "xT_loc": xT_loc, "eaS": eaS,
             "kvidx": kvidx, "qdidx": qdidx}
        m.update(shared)
        in_maps.append(m)
        needs_b_all.append(needs_b)
    chunk_b = tuple(bool(v) for v in np.logical_or.reduce(needs_b_all))
    return in_maps, K, T, newid, chunk_b


def _build(K, T, chunk_b):
    NOCC = int(os.environ.get("KNOCC", "0"))
    nc = bacc.Bacc("TRN2", target_bir_lowering=False, debug=False,
                   enable_asserts=False, num_devices=C, num_swdge_queues=4)
    tile_block = np.repeat(np.arange(NBLK), K)
    blk_tile_start = np.concatenate([[0], np.cumsum(K)])[:NBLK]
    NCH = T // G
    IW = P // 16                  # idx columns per tile

    # ------------- I/O -------------
    x_in = nc.dram_tensor("x_loc", [NL, D], f32, kind="ExternalInput")
    xT_in = nc.dram_tensor("xT_loc", [D, NL], bf16, kind="ExternalInput")
    eaS_in = nc.dram_tensor("eaS", [T, P, 4, P], bf16, kind="ExternalInput")
    kvidx_in = nc.dram_tensor("kvidx", [P, T * IW], i16, kind="ExternalInput")
    qdidx_in = nc.dram_tensor("qdidx", [P, T * IW], i16, kind="ExternalInput")
    wq_in = nc.dram_tensor("Wq", [4, P, D], bf16, kind="ExternalInput")
    wk_in = nc.dram_tensor("Wk", [4, P, D], bf16, kind="ExternalInput")
    wv_in = nc.dram_tensor("Wv", [4, P, D], bf16, kind="ExternalInput")
    wsk_in = nc.dram_tensor("Wsk", [4, P, D], bf16, kind="ExternalInput")
    we_in = nc.dram_tensor("We", [3, P, D], bf16, kind="ExternalInput")
    w1_in = nc.dram_tensor("W1", [3, P, DH], bf16, kind="ExternalInput")
    bm1_in = nc.dram_tensor("bm1T", [P, H], f32, kind="ExternalInput")
    w2_in = nc.dram_tensor("W2", [6, P, D], bf16, kind="ExternalInput")
    bm2_in = nc.dram_tensor("bm2", [1, D], bf16, kind="ExternalInput")
    gb_in = nc.dram_tensor("gb", [3, P, 4], f32, kind="ExternalInput")
    # output is written feature-major [D, NL]; the host transposes it back
    out_dram = nc.dram_tensor("out", [D, NL], f32, kind="ExternalOutput")

    # ------------- internal DRAM -------------
    kv_part = nc.dram_tensor("kv_part", [NL, 2 * D], f8)
    kv_full = nc.dram_tensor("kv_full", [C * NL, 2 * D], f8,
                             addr_space="Shared")
    q_part = nc.dram_tensor("q_part", [NL, D], bf16)
    bn2_i = nc.dram_tensor("bn2_i", [P, 6], f32)
    bn2_o = nc.dram_tensor("bn2_o", [P, 6], f32, addr_space="Shared")

    rg = [list(range(C))]

    with tile.TileContext(nc) as tc:
        with tc.tile_pool(name="const", bufs=1) as cp, \
             tc.tile_pool(name="persist", bufs=1) as pp, \
             tc.tile_pool(name="small", bufs=2) as sp:
            # ---- constants ----
            idn_f = cp.tile([P, P], f32, tag="idn_f")
            make_identity(nc, idn_f[:])
            idn8 = cp.tile([P, P], f8, tag="idn8")
            make_identity(nc, idn8[:])
            ones_b = cp.tile([1, NL], bf16, tag="ones_b")
            nc.vector.memset(ones_b[:], 1.0)
            epst = cp.tile([P, 1], f32, tag="epst")
            nc.vector.memset(epst[:], EPS)
            wq_sb = cp.tile([P, 4, D], bf16, tag="wq")
            nc.sync.dma_start(wq_sb[:], wq_in.ap().rearrange("c p d -> p c d"))
            wk_sb = cp.tile([P, 4, D], bf16, tag="wk")
            nc.sync.dma_start(wk_sb[:], wk_in.ap().rearrange("c p d -> p c d"))
            wv_sb = cp.tile([P, 4, D], bf16, tag="wv")
            nc.sync.dma_start(wv_sb[:], wv_in.ap().rearrange("c p d -> p c d"))
            wsk_sb = cp.tile([P, 4, D], bf16, tag="wsk")
            nc.sync.dma_start(wsk_sb[:], wsk_in.ap().rearrange("c p d -> p c d"))
            we_sb = cp.tile([P, 3, D], bf16, tag="we")
            nc.sync.dma_start(we_sb[:], we_in.ap().rearrange("c p d -> p c d"))
            gb_sb = cp.tile([P, 3, 4], f32, tag="gb")
            nc.sync.dma_start(gb_sb[:], gb_in.ap().rearrange("c p j -> p c j"))

            # xT (feature-major input, bf16) — BN1 is folded into the
            # projection weights, so this feeds the projections directly.
            hp_cm = tc.tile_pool(name="xtp", bufs=1)
            hp = hp_cm.__enter__()
            xT = [hp.tile([P, NL], bf16, tag=f"xT{c}", name=f"xT{c}")
                  for c in range(3)]
            NQ = NL // 4
            for c in range(3):
                for h4 in range(4):
                    cs = slice(h4 * NQ, (h4 + 1) * NQ)
                    nc.sync.dma_start(xT[c][:, cs],
                                      xT_in[c * P:(c + 1) * P, cs])

            def bn_affine(get, gcol, bcol, scn, bin_):
                """From AllReduced per-feature (sum, sumsq) compute
                scale/bias [P, 3] tiles."""
                sc_t = cp.tile([P, 3], f32, tag=scn)
                bi_t = cp.tile([P, 3], f32, tag=bin_)
                for c in range(3):
                    mean = sp.tile([P, 1], f32, tag="bn_mean")
                    nc.vector.tensor_scalar_mul(mean[:], get(c, 0), 1.0 / N)
                    var = sp.tile([P, 1], f32, tag="bn_var")
                    nc.vector.tensor_scalar_mul(var[:], get(c, 1), 1.0 / N)
                    msq = sp.tile([P, 1], f32, tag="bn_msq")
                    nc.vector.tensor_tensor(out=msq[:], in0=mean[:], in1=mean[:],
                                            op=ALU.mult)
                    nc.vector.tensor_tensor(out=var[:], in0=var[:], in1=msq[:],
                                            op=ALU.subtract)
                    std = sp.tile([P, 1], f32, tag="bn_std")
                    nc.scalar.activation(std[:], var[:], AF.Sqrt, bias=epst[:, 0:1])
                    rstd = sp.tile([P, 1], f32, tag="bn_rstd")
                    nc.vector.reciprocal(rstd[:], std[:])
                    nc.vector.tensor_tensor(out=sc_t[:, c:c + 1], in0=rstd[:],
                                            in1=gb_sb[:, c, gcol:gcol + 1],
                                            op=ALU.mult)
                    ms = sp.tile([P, 1], f32, tag="bn_ms")
                    nc.vector.tensor_tensor(out=ms[:], in0=mean[:],
                                            in1=sc_t[:, c:c + 1], op=ALU.mult)
                    nc.vector.tensor_tensor(out=bi_t[:, c:c + 1],
                                            in0=gb_sb[:, c, bcol:bcol + 1],
                                            in1=ms[:], op=ALU.subtract)
                return sc_t, bi_t

            # ---- projections: k,v first (feeds chunked AllGather), then
            # q (to DRAM, pre-scaled) and skip; base = x + skip ----
            base = [pp.tile([P, D], f32, tag=f"base{b}", name=f"base{b}")
                    for b in range(NBLK)]
            with tc.tile_pool(name="proj", bufs=3) as jp, \
                 tc.tile_pool(name="projps", bufs=2, space="PSUM") as jpp:

                def proj_ps(b, wsb):
                    pb = _block_pb(b)
                    ns = slice(b * P, b * P + pb)
                    ps = jpp.tile([P, D], f32, tag="proj_ps", space="PSUM")
                    for kc in range(3):
                        nc.tensor.matmul(ps[:pb, :], lhsT=xT[kc][:, ns],
                                         rhs=wsb[:, kc, :],
                                         start=(kc == 0), stop=False,
                                         skip_group_check=True)
                    nc.tensor.matmul(ps[:pb, :], lhsT=ones_b[:, ns],
                                     rhs=wsb[0:1, 3, :],
                                     start=False, stop=True,
                                     skip_group_check=True)
                    return ps, pb, ns

                def kv_block(b):
                    # kv rows hold [k | v-k] so the attention phase can build
                    # both ke and ve by PE accumulation onto the shared e PSUM
                    ps_k, pb, ns = proj_ps(b, wk_sb)
                    ob_k = jp.tile([P, D], f8, tag="kv_out")
                    nc.vector.tensor_copy(ob_k[:pb, :], ps_k[:pb, :])
                    nc.sync.dma_start(kv_part[ns, 0:D], ob_k[:pb, :])
                    ps_v, pb, ns = proj_ps(b, wv_sb)
                    ob_vmk = jp.tile([P, D], f8, tag="kv_out2")
                    nc.vector.tensor_tensor(out=ob_vmk[:pb, :],
                                            in0=ps_v[:pb, :],
                                            in1=ob_k[:pb, :],
                                            op=ALU.subtract)
                    nc.sync.dma_start(kv_part[ns, D:2 * D], ob_vmk[:pb, :])

                if KAG2:
                    for b in range(10):
                        kv_block(b)
                    if NOCC:
                        for cc in range(C):
                            nc.sync.dma_start(
                                kv_full[cc * SPLIT:(cc + 1) * SPLIT, :],
                                kv_part[0:SPLIT, :])
                    else:
                        nc.gpsimd.collective_compute(
                            "AllGather", ALU.bypass, replica_groups=rg,
                            ins=[kv_part[0:SPLIT, :].opt()],
                            outs=[kv_full[0:C * SPLIT, :].opt()])
                    for b in range(10, NBLK):
                        kv_block(b)
                    if NOCC:
                        for cc in range(C):
                            nc.sync.dma_start(
                                kv_full[C * SPLIT + cc * REST:
                                        C * SPLIT + (cc + 1) * REST, :],
                                kv_part[SPLIT:NL, :])
                    else:
                        nc.gpsimd.collective_compute(
                            "AllGather", ALU.bypass, replica_groups=rg,
                            ins=[kv_part[SPLIT:NL, :].opt()],
                            outs=[kv_full[C * SPLIT:C * NL, :].opt()])
                else:
                    for b in range(NBLK):
                        kv_block(b)
                    if NOCC:
                        for cc in range(C):
                            nc.sync.dma_start(
                                kv_full[cc * NL:(cc + 1) * NL, :],
                                kv_part[:, :])
                    else:
                        nc.gpsimd.collective_compute(
                            "AllGather", ALU.bypass, replica_groups=rg,
                            ins=[kv_part.ap().opt()],
                            outs=[kv_full.ap().opt()])

                for b in range(NBLK):
                    ps, pb, ns = proj_ps(b, wq_sb)
                    qb = jp.tile([P, D], bf16, tag="proj_out")
                    nc.vector.tensor_copy(qb[:pb, :], ps[:pb, :])
                    nc.sync.dma_start(q_part[ns, :], qb[:pb, :])
                for b in range(NBLK):
                    ps, pb, ns = proj_ps(b, wsk_sb)
                    xb = jp.tile([P, D], f32, tag="xb")
                    nc.sync.dma_start(xb[:pb, :], x_in[ns, :])
                    nc.vector.tensor_tensor(out=base[b][:pb, :],
                                            in0=xb[:pb, :],
                                            in1=ps[:pb, :], op=ALU.add)

            hp_cm.__exit__(None, None, None)

            # x2 transposed (feature-major, f32 so it can carry the final
            # residual) — filled as blocks finalize, with BN2 sums
            # accumulated for free on the ACT copies.
            x2T = [pp.tile([P, NL], f32, tag=f"x2T{c}", name=f"x2T{c}")
                   for c in range(3)]
            bn2acc = cp.tile([P, 3, NBLK], f32, tag="bn2acc")
            bn2sq = cp.tile([P, 3, NBLK], f32, tag="bn2sq")
            kvidx_sb = cp.tile([P, T * IW], i16, tag="kvidx")
            nc.sync.dma_start(kvidx_sb[:], kvidx_in[:, :])
            qdidx_sb = cp.tile([P, T * IW], i16, tag="qdidx")
            nc.sync.dma_start(qdidx_sb[:], qdidx_in[:, :])

            # ---- attention over edge tiles ----
            with tc.tile_pool(name="attg", bufs=4) as gp, \
                 tc.tile_pool(name="attS", bufs=12) as sp2, \
                 tc.tile_pool(name="attw", bufs=4) as aw, \
                 tc.tile_pool(name="atte", bufs=4) as ep, \
                 tc.tile_pool(name="attps", bufs=2, space="PSUM") as pps, \
                 tc.tile_pool(name="aggps", bufs=2, space="PSUM") as agp, \
                 tc.tile_pool(name="fin", bufs=3) as fp:
                agg_ps = {}

                def finalize_block(b, ag):
                    """attn = agg/denom; base += attn; transpose base into
                    x2T accumulating BN2 sums."""
                    pb = _block_pb(b)
                    dn = fp.tile([P, H], f32, tag="dn")
                    nc.vector.tensor_scalar_max(dn[:], ag[:, D:D + H],
                                                1e-30)
                    rd = fp.tile([P, H], f32, tag="rd")
                    nc.vector.reciprocal(rd[:], dn[:])
                    at = fp.tile([P, D], f32, tag="at")
                    nc.vector.scalar_tensor_tensor(
                        out=at[:pb].rearrange("p (d h) -> p d h", h=H),
                        in0=ag[:pb, 0:D].rearrange(
                            "p (d h) -> p d h", h=H),
                        scalar=1.0,
                        in1=rd[:pb, None, :].to_broadcast(
                            [pb, DHEAD, H]),
                        op0=ALU.bypass, op1=ALU.mult)
                    nc.vector.tensor_tensor(
                        out=base[b][:pb].rearrange(
                            "p (h d) -> p h d", h=H),
                        in0=base[b][:pb].rearrange(
                            "p (h d) -> p h d", h=H),
                        in1=at[:pb].rearrange("p (d h) -> p h d", h=H),
                        op=ALU.add)
                    # x2T + BN2 sum/sumsq accum (same PSUM ring as agg)
                    ns = slice(b * P, b * P + pb)
                    for dc in range(3):
                        tp_ps = agp.tile([P, D + H], f32, tag="agg",
                                         name=f"tp{b}_{dc}", space="PSUM")
                        nc.tensor.transpose(
                            out=tp_ps[:, :pb],
                            in_=base[b][:pb, dc * P:(dc + 1) * P],
                            identity=idn_f[:pb, :pb])
                        nc.scalar.activation(
                            x2T[dc][:, ns], tp_ps[:, :pb], AF.Identity,
                            accum_out=bn2acc[:, dc, b:b + 1])
                        sqs = fp.tile([P, P], bf16, tag="sqs")
                        nc.scalar.activation(
                            sqs[:, :pb], tp_ps[:, :pb], AF.Square,
                            accum_out=bn2sq[:, dc, b:b + 1])

                def stage_b(t0, S_tiles, ps_e, rhs):
                    # wve = (v[src]+e) * w; ve sits in PSUM (e + k + (v-k)
                    # accumulated by PE); route through an ACT copy so the
                    # DVE multiply runs in bf16 2x mode.
                    ve_sb = ep.tile([P, TF, D], bf16, tag="vesb")
                    nc.scalar.activation(ve_sb[:], ps_e[:, :, 0:D],
                                         AF.Identity)
                    nc.vector.tensor_tensor(
                        out=rhs[:, :, 0:D].rearrange(
                            "p t (d h) -> p t d h", h=H),
                        in0=ve_sb[:, :, :].rearrange(
                            "p t (d h) -> p t d h", h=H),
                        in1=rhs[:, :, None, D:D + H].to_broadcast(
                            [P, TF, DHEAD, H]),
                        op=ALU.mult)
                    # aggregate into per-block PSUM accumulators
                    for j in range(TF):
                        t = t0 + j
                        b = int(tile_block[t])
                        first = (t == blk_tile_start[b])
                        last = (t == blk_tile_start[b] + K[b] - 1)
                        if first:
                            agg_ps[b] = agp.tile([P, D + H], f32, tag="agg",
                                                 name=f"agg{b}", space="PSUM")
                        nc.tensor.matmul(agg_ps[b][:, :],
                                         lhsT=S_tiles[j][:, 3, :],
                                         rhs=rhs[:, j, :], start=first,
                                         stop=last, skip_group_check=True)
                        if last:
                            finalize_block(b, agg_ps.pop(b))

                pending = None
                NQPRE = 2          # q gathers issued ahead (run under the AG)
                qsrcs = {}
                qq = [0]

                def q_gather(c):
                    if c >= NCH:
                        return
                    qsrcs[c] = gp.tile([P, G, D], bf16, tag="qsrc", bufs=3,
                                       name=f"qsrc{c}")
                    nc.gpsimd.dma_gather(
                        out_ap=qsrcs[c][:, :, :], in_ap=q_part[:, :],
                        idxs_ap=qdidx_sb[:, c * G * IW:(c + 1) * G * IW],
                        num_idxs=G * P, num_idxs_reg=G * P, elem_size=D,
                        queue_num=qq[0] % 4)
                    qq[0] += 1

                for c in range(NQPRE):
                    q_gather(c)
                for ch in range(NCH):
                    kvsrc = gp.tile([P, G, 2 * D], f8, tag="kvsrc",
                                    bufs=4)
                    kv_src_ap = (kv_full[:, :] if (not KAG2 or chunk_b[ch])
                                 else kv_full[0:C * SPLIT, :])
                    nc.gpsimd.dma_gather(
                        out_ap=kvsrc[:, :, :], in_ap=kv_src_ap,
                        idxs_ap=kvidx_sb[:, ch * G * IW:(ch + 1) * G * IW],
                        num_idxs=G * P, num_idxs_reg=G * P, elem_size=2 * D,
                        queue_num=qq[0] % 4)
                    qq[0] += 1
                    q_gather(ch + NQPRE)
                    qsrc = qsrcs.pop(ch)
                    for s2 in range(G // TF):
                        j0 = s2 * TF
                        t0 = ch * G + j0
                        ps_e = pps.tile([P, TF, 512], f32, tag="ps_e",
                                        space="PSUM")
                        S_tiles = []
                        for j in range(TF):
                            t = t0 + j
                            ea_t = sp2.tile([P, 4, P], bf16, tag="ea")
                            nc.sync.dma_start(ea_t[:], eaS_in[t, :, :, :])
                            S_tiles.append(ea_t)
                            for kc in range(3):
                                nc.tensor.matmul(ps_e[:, j, 0:D],
                                                 lhsT=ea_t[:, kc, :],
                                                 rhs=we_sb[:, kc, :],
                                                 start=(kc == 0), stop=False,
                                                 skip_group_check=True)
                        for j in range(TF):
                            nc.tensor.matmul(ps_e[:, j, 0:D], lhsT=idn8[:],
                                             rhs=kvsrc[:, j0 + j, 0:D],
                                             start=False, stop=False,
                                             skip_group_check=True)
                        prod = aw.tile([P, TF, D], bf16, tag="prod")
                        nc.vector.tensor_tensor(out=prod[:],
                                                in0=qsrc[:, j0:j0 + TF, :],
                                                in1=ps_e[:, :, 0:D],
                                                op=ALU.mult)
                        for j in range(TF):
                            nc.tensor.matmul(ps_e[:, j, 0:D], lhsT=idn8[:],
                                             rhs=kvsrc[:, j0 + j, D:2 * D],
                                             start=False, stop=True,
                                             skip_group_check=True)
                        fold = ep.tile([P, TF, 32 * H], bf16, tag="fold")
                        nc.vector.tensor_tensor(
                            out=fold[:], in0=prod[:, :, 0:32 * H],
                            in1=prod[:, :, 32 * H:64 * H], op=ALU.add)
                        fold2 = ep.tile([P, TF, 16 * H], bf16, tag="fold2")
                        nc.vector.tensor_tensor(
                            out=fold2[:], in0=fold[:, :, 0:16 * H],
                            in1=fold[:, :, 16 * H:32 * H], op=ALU.add)
                        lg = ep.tile([P, TF, H], f32, tag="lg")
                        nc.vector.tensor_reduce(
                            out=lg[:],
                            in_=fold2[:].rearrange("p t (d h) -> p t h d", h=H),
                            axis=mybir.AxisListType.X, op=ALU.add)
                        rhs = aw.tile([P, TF, D + H], bf16, tag="rhs")
                        nc.scalar.activation(rhs[:, :, D:D + H], lg[:], AF.Exp)
                        if pending is not None:
                            stage_b(*pending)
                        pending = (t0, S_tiles, ps_e, rhs)
                if pending is not None:
                    stage_b(*pending)

            # ---- BN2 stats: sums/sumsq accumulated during attention
            bn2_sb = cp.tile([P, 6], f32, tag="bn2sb")
            xp_cm = tc.tile_pool(name="x2sq", bufs=2)
            xp = xp_cm.__enter__()
            nc.vector.tensor_reduce(
                out=bn2_sb[:, 0:3].rearrange("p (c o) -> p c o", o=1),
                in_=bn2acc[:, :, :],
                axis=mybir.AxisListType.X, op=ALU.add)
            nc.vector.tensor_reduce(
                out=bn2_sb[:, 3:6].rearrange("p (c o) -> p c o", o=1),
                in_=bn2sq[:, :, :],
                axis=mybir.AxisListType.X, op=ALU.add)
            nc.sync.dma_start(bn2_i[:, :], bn2_sb[:])
            if NOCC:
                nc.sync.dma_start(bn2_o[:, :], bn2_i[:, :])
            else:
                nc.gpsimd.collective_compute(
                    "AllReduce", ALU.add, replica_groups=rg,
                    ins=[bn2_i.ap().opt()], outs=[bn2_o.ap().opt()])
            st2 = cp.tile([P, 6], f32, tag="st2")
            nc.sync.dma_start(st2[:], bn2_o[:, :])
            sc2, bi2 = bn_affine(lambda c, k: st2[:, c + 3 * k:c + 3 * k + 1],
                                 2, 3, "sc2", "bi2")

            h2T = [xp.tile([P, NL], bf16, tag=f"h2T{c}", name=f"h2T{c}",
                           bufs=1)
                   for c in range(3)]
            for c in range(3):
                nc.scalar.activation(h2T[c][:], x2T[c][:], AF.Identity,
                                     scale=sc2[:, c:c + 1], bias=bi2[:, c:c + 1])

            # ---- MLP (transposed) + residual + output; 4-block supers ----
            with tc.tile_pool(name="mlpw", bufs=1) as mwp, \
                 tc.tile_pool(name="mlp", bufs=3) as mp, \
                 tc.tile_pool(name="mlpps", bufs=2, space="PSUM") as mpp:
                w1_sb = mwp.tile([P, 3, DH], bf16, tag="w1")
                nc.sync.dma_start(w1_sb[:],
                                  w1_in.ap().rearrange("c p d -> p c d"))
                bm1_sb = mwp.tile([P, H], f32, tag="bm1")
                nc.sync.dma_start(bm1_sb[:], bm1_in[:, :])
                w2_sb = mwp.tile([P, 6, D], bf16, tag="w2")
                nc.sync.dma_start(w2_sb[:],
                                  w2_in.ap().rearrange("c p d -> p c d"))
                bm2_sb = mwp.tile([1, D], bf16, tag="bm2")
                nc.sync.dma_start(bm2_sb[:], bm2_in[:, :])
                for b0 in range(0, NBLK, 4):
                    bs = [b0, b0 + 1, b0 + 2, b0 + 3]
                    pbs = [_block_pb(b) for b in bs]
                    pb4 = sum(pbs)
                    ns2 = slice(b0 * P, b0 * P + pb4)
                    gT = []
                    for oc in range(H):
                        m1 = mpp.tile([P, 4 * P], f32, tag="m1", space="PSUM")
                        for kc in range(3):
                            nc.tensor.matmul(
                                m1[:, :pb4],
                                lhsT=w1_sb[:, kc, oc * P:(oc + 1) * P],
                                rhs=h2T[kc][:, ns2], start=(kc == 0),
                                stop=(kc == 2), skip_group_check=True)
                        g_t = mp.tile([P, 4 * P], bf16, tag=f"gT{oc}")
                        nc.scalar.activation(g_t[:, :pb4], m1[:, :pb4], AF.Gelu,
                                             bias=bm1_sb[:, oc:oc + 1])
                        gT.append(g_t)
                    for dc in range(3):
                        m2 = mpp.tile([P, 4 * P], f32, tag="m2", space="PSUM")
                        for oc in range(H):
                            nc.tensor.matmul(
                                m2[:, :pb4],
                                lhsT=w2_sb[:, oc, dc * P:(dc + 1) * P],
                                rhs=gT[oc][:, :pb4], start=(oc == 0), stop=False,
                                skip_group_check=True)
                        nc.tensor.matmul(m2[:, :pb4],
                                         lhsT=bm2_sb[0:1, dc * P:(dc + 1) * P],
                                         rhs=ones_b[:, ns2], start=False,
                                         stop=True, skip_group_check=True)
                        outT = mp.tile([P, 4 * P], f32, tag="outT")
                        nc.vector.tensor_tensor(
                            out=outT[:, :pb4], in0=x2T[dc][:, ns2],
                            in1=m2[:, :pb4], op=ALU.add)
                        nc.sync.dma_start(
                            out_dram[dc * P:(dc + 1) * P, ns2],
                            outT[:, :pb4])
            xp_cm.__exit__(None, None, None)
    nc.compile()
    return nc


_CACHE = {}


def kernel(x, edge_index, edge_attr, g1, b1, Wq, bq, Wk, bk, Wv, bv, We,
           Wskip, bskip, g2, b2, W1, bm1, W2, bm2):
    weights = (np.asarray(Wq, np.float32), np.asarray(bq, np.float32),
               np.asarray(Wk, np.float32), np.asarray(bk, np.float32),
               np.asarray(Wv, np.float32), np.asarray(bv, np.float32),
               np.asarray(We, np.float32),
               np.asarray(Wskip, np.float32), np.asarray(bskip, np.float32),
               np.asarray(g1, np.float32), np.asarray(b1, np.float32),
               np.asarray(g2, np.float32), np.asarray(b2, np.float32),
               np.asarray(W1, np.float32), np.asarray(bm1, np.float32),
               np.asarray(W2, np.float32), np.asarray(bm2, np.float32))
    in_maps, K, T, newid, chunk_b = _prep_host(x, edge_index, edge_attr,
                                               weights)
    key = (tuple(K), chunk_b)
    if key not in _CACHE:
        _CACHE[key] = _build(K, T, chunk_b)
    nc = _CACHE[key]
    res = run_bass_kernel_spmd(nc, in_maps, core_ids=list(range(C)))
    out = np.concatenate([res.results[c]["out"].T for c in range(C)], axis=0)
    return out[newid].astype(np.float32)


if __name__ == "__main__":
    import reference
    inputs = {k: np.asarray(v) for k, v in reference.setup_inputs().items()}
    got = kernel(**inputs)
    exp = np.asarray(reference.reference(**inputs))
    num = np.linalg.norm((got - exp).astype(np.float64))
    den = np.linalg.norm(exp.astype(np.float64))
    print("Relative error:", num / den)
